# revision 8
# baseline (speedup 1.0000x reference)
"""Bipartite graph convolution (GCMC-style) Trainium2 kernel, 8-core SPMD.

Math (reference): per-rating masks M_r = (adj == r), r=1..5,
  out_u = relu(d_u * sum_r (M_r @ v_feat) @ W_u[r]),  d_u = 1/deg_u
  out_v = relu(d_v * sum_r (M_r.T @ u_feat) @ W_v[r]), d_v = 1/deg_v

Device formulation (per core, u-rows sharded 1024/core), v2:
  Fold weights on host: P_r = v_feat @ W_u[r], Q_r = u_shard @ W_v[r].
  Step basis: {a, s2, s3, s4, s5} with s_c(a) = (a >= c); host solves
  P^_k so that sum_k g_k(a) P^_k == P_a for a in 0..5 (0 -> 0).
  The masks/steps are the MOVING matmul operand (long 512-col streams)
  and the 64-wide features are stationary; two independent M=64 matmuls
  run concurrently in the two column halves of the PE array
  (tile_position col groups), so the array is fully utilized:
    group A (psum partitions 0:64)  <- even chunk of the pair
    group B (psum partitions 64:128) <- odd chunk of the pair
  Host sums the two halves afterwards (plus cross-core all-reduce for
  out_v), applies 1/deg scaling and relu.
  Step planes are generated on-chip from the streamed adj tiles:
  DVE is_ge for most planes, ACT sigmoid(40*(a-c+0.5)) for ~1 plane per
  position to balance engine load (both give exact 0/1 in fp16).
"""

import numpy as np
import sys

sys.path.insert(0, "/opt/trn_rl_repo")

N_U, N_V = 8192, 8192
F = 64
R = 5
N_CORES = 8
U_SH = N_U // N_CORES          # 1024 rows per core
UC = U_SH // 128               # 8 u-chunks per core
VC = N_V // 128                # 64 v-chunks
WB = 2048                      # phase-B v-window width
NWIN = N_V // WB               # 4 windows

_CACHE = {}

# engine split: positions with (index % ACT_SKIP == ACT_SKIP-1) generate
# all 4 step planes on DVE; the rest do 3 on DVE + threshold-4 on ACT.
ACT_SKIP = 8


def _build():
    import concourse.bass as bass
    import concourse.bacc as bacc
    import concourse.mybir as mybir
    import concourse.tile as tile

    dt = mybir.dt
    ge = mybir.AluOpType.is_ge
    SIG = mybir.ActivationFunctionType.Sigmoid

    nc = bacc.Bacc("TRN2", target_bir_lowering=False, debug=False,
                   num_devices=N_CORES)

    adjt_h = nc.dram_tensor("adjt_h", [N_V, U_SH], dt.float16,
                            kind="ExternalInput").ap()
    adj_h = nc.dram_tensor("adj_h", [U_SH, N_V], dt.float16,
                           kind="ExternalInput").ap()
    p_hat_h = nc.dram_tensor("p_hat_h", [128, VC * R * F], dt.float16,
                             kind="ExternalInput").ap()
    q_hat_h = nc.dram_tensor("q_hat_h", [128, UC * R * F], dt.float16,
                             kind="ExternalInput").ap()
    out_ut = nc.dram_tensor("out_ut", [128, U_SH], dt.float16,
                            kind="ExternalOutput").ap()
    out_vt = nc.dram_tensor("out_vt", [128, N_V], dt.float16,
                            kind="ExternalOutput").ap()

    def gen_planes(nc, pl, src, W, all_dve, bias_s4):
        """Write step planes s2..s5 of src into quarters of pl [128, 4W].
        Threshold c=k+2 for quarter k. DVE is_ge for all but k=2, which
        goes to ACT (sigmoid step) unless all_dve."""
        for k in range(4):
            c = k + 2
            dst = pl[:, k * W:(k + 1) * W]
            if k == 2 and not all_dve:
                # sigmoid(40*(a - (c-0.5))): exact 0/1 in fp16 for ints
                nc.scalar.activation(dst, src, SIG,
                                     bias=bias_s4[:, 0:1], scale=40.0)
            else:
                nc.vector.tensor_scalar(dst, src, float(c), None, op0=ge)

    with tile.TileContext(nc) as tc:
        with tc.tile_pool(name="consts", bufs=1) as cons, \
             tc.tile_pool(name="streamA", bufs=4) as streamA, \
             tc.tile_pool(name="planesA", bufs=3) as planesA, \
             tc.tile_pool(name="streamB", bufs=4) as streamB, \
             tc.tile_pool(name="planesB", bufs=3) as planesB, \
             tc.tile_pool(name="fin", bufs=4) as fin:

            p_hat = cons.tile([128, VC * R * F], dt.float16, tag="ph")
            q_hat = cons.tile([128, UC * R * F], dt.float16, tag="qh")
            # chunked so early matmuls only wait for their slice
            PCH = VC * R * F // 16
            for i in range(16):
                nc.sync.dma_start(p_hat[:, i * PCH:(i + 1) * PCH],
                                  p_hat_h[:, i * PCH:(i + 1) * PCH])
            nc.sync.dma_start(q_hat[:], q_hat_h[:])
            bias_s4 = cons.tile([128, 1], dt.float32, tag="bs4")
            nc.gpsimd.memset(bias_s4[:], -40.0 * 3.5)

            # ---------------- Phase A: out_uT ----------------
            pspA = tc.tile_pool(name="psumA", bufs=1, space="PSUM")
            psA = pspA.__enter__()
            ps_u = psA.tile([128, U_SH], dt.float32, tag="psu")
            pos = 0
            for t in range(VC // 2):
                srcs = []
                planes = []
                for j in (0, 1):
                    vc = 2 * t + j
                    at = streamA.tile([128, U_SH], dt.float16, tag="adjt",
                                      name=f"at{vc}")
                    nc.sync.dma_start(at[:],
                                      adjt_h[vc * 128:(vc + 1) * 128, :])
                    pl = planesA.tile([128, 4 * U_SH], dt.float16,
                                      tag="plA", name=f"plA{vc}")
                    all_dve = (pos % ACT_SKIP) == ACT_SKIP - 1
                    pos += 1
                    gen_planes(nc, pl, at[:], U_SH, all_dve, bias_s4)
                    srcs.append(at)
                    planes.append(pl)
                for b in range(R):
                    for j in (0, 1):
                        for n in range(U_SH // 512):
                            vc = 2 * t + j
                            if b == 0:
                                mov = srcs[j][:, n * 512:(n + 1) * 512]
                            else:
                                off = (b - 1) * U_SH + n * 512
                                mov = planes[j][:, off:off + 512]
                            nc.tensor.matmul(
                                ps_u[64 * j:64 * (j + 1),
                                     n * 512:(n + 1) * 512],
                                p_hat[:, (vc * R + b) * F:
                                      (vc * R + b + 1) * F],
                                mov,
                                start=(t == 0 and b == 0),
                                stop=(t == VC // 2 - 1 and b == R - 1),
                                skip_group_check=True)
            # evacuate out_uT (raw; host applies deg/relu/transpose)
            for n in range(U_SH // 512):
                ev = fin.tile([128, 512], dt.float16, tag="evu",
                              name=f"evu{n}")
                if n % 2 == 0:
                    nc.scalar.copy(ev[:], ps_u[:, n * 512:(n + 1) * 512])
                else:
                    nc.vector.tensor_copy(ev[:], ps_u[:, n * 512:(n + 1) * 512])
                nc.sync.dma_start(out_ut[:, n * 512:(n + 1) * 512], ev[:])
            pspA.__exit__(None, None, None)

            # ---------------- Phase B: out_vT ----------------
            pspB = tc.tile_pool(name="psumB", bufs=1, space="PSUM")
            psB = pspB.__enter__()
            pos = 0
            for w in range(NWIN):
                psv = psB.tile([128, WB], dt.float32, tag="psv", bufs=2,
                               name=f"psv{w}")
                for up in range(UC // 2):
                    srcs = []
                    planes = []
                    for j in (0, 1):
                        uc = 2 * up + j
                        ad = streamB.tile([128, WB], dt.float16, tag="adjb",
                                          name=f"ad{w}_{uc}")
                        nc.sync.dma_start(
                            ad[:], adj_h[uc * 128:(uc + 1) * 128,
                                         w * WB:(w + 1) * WB])
                        pl = planesB.tile([128, 4 * WB], dt.float16,
                                          tag="plB", name=f"plB{w}_{uc}")
                        all_dve = (pos % ACT_SKIP) == ACT_SKIP - 1
                        pos += 1
                        gen_planes(nc, pl, ad[:], WB, all_dve, bias_s4)
                        srcs.append(ad)
                        planes.append(pl)
                    for b in range(R):
                        for j in (0, 1):
                            for n in range(WB // 512):
                                uc = 2 * up + j
                                if b == 0:
                                    mov = srcs[j][:, n * 512:(n + 1) * 512]
                                else:
                                    off = (b - 1) * WB + n * 512
                                    mov = planes[j][:, off:off + 512]
                                nc.tensor.matmul(
                                    psv[64 * j:64 * (j + 1),
                                        n * 512:(n + 1) * 512],
                                    q_hat[:, (uc * R + b) * F:
                                          (uc * R + b + 1) * F],
                                    mov,
                                    start=(up == 0 and b == 0),
                                    stop=(up == UC // 2 - 1 and b == R - 1),
                                    skip_group_check=True)
                for n in range(WB // 512):
                    ev = fin.tile([128, 512], dt.float16, tag="evv",
                                  name=f"evv{w}_{n}")
                    if n % 2 == 0:
                        nc.scalar.copy(ev[:], psv[:, n * 512:(n + 1) * 512])
                    else:
                        nc.vector.tensor_copy(ev[:],
                                              psv[:, n * 512:(n + 1) * 512])
                    nc.sync.dma_start(
                        out_vt[:, w * WB + n * 512:w * WB + (n + 1) * 512],
                        ev[:])
            pspB.__exit__(None, None, None)

    nc.compile()
    return nc


def _basis_matrix():
    # rows r=1..5, cols k: [a, s2, s3, s4, s5]
    M = np.zeros((R, R))
    for r in range(1, R + 1):
        M[r - 1, 0] = r
        for k in range(1, R):
            M[r - 1, k] = 1.0 if r >= k + 1 else 0.0
    return M


def _host_prep(adj, u_feature, v_feature, weight_u, weight_v):
    adj = np.asarray(adj)
    u_feature = np.asarray(u_feature, dtype=np.float32)
    v_feature = np.asarray(v_feature, dtype=np.float32)
    weight_u = np.asarray(weight_u, dtype=np.float32)
    weight_v = np.asarray(weight_v, dtype=np.float32)

    adj16 = adj.astype(np.float16)
    Minv = np.linalg.inv(_basis_matrix())

    # P_r = v_feat @ W_u[r]; basis-transform to step basis
    P = np.einsum("vf,rfo->rvo", v_feature, weight_u)       # [R, N_V, F]
    Pb = np.tensordot(Minv, P, axes=([1], [0]))             # [R, N_V, F]
    # p_hat[p, (vc*R+b)*F + f] = Pb[b, vc*128+p, f]
    p_hat = np.ascontiguousarray(
        Pb.reshape(R, VC, 128, F).transpose(2, 1, 0, 3).reshape(128, -1)
    ).astype(np.float16)

    in_maps = []
    for c in range(N_CORES):
        sl = slice(c * U_SH, (c + 1) * U_SH)
        Q = np.einsum("uf,rfo->ruo", u_feature[sl], weight_v)  # [R, U_SH, F]
        Qb = np.tensordot(Minv, Q, axes=([1], [0]))
        q_hat = np.ascontiguousarray(
            Qb.reshape(R, UC, 128, F).transpose(2, 1, 0, 3).reshape(128, -1)
        ).astype(np.float16)
        a = adj16[sl]
        in_maps.append({
            "adj_h": np.ascontiguousarray(a),
            "adjt_h": np.ascontiguousarray(a.T),
            "p_hat_h": p_hat,
            "q_hat_h": q_hat,
        })
    return in_maps


def kernel(adj, u_feature, v_feature, weight_u, weight_v, _trace=False):
    from concourse import bass_utils

    if "nc" not in _CACHE:
        _CACHE["nc"] = _build()
    nc = _CACHE["nc"]

    adj = np.asarray(adj)
    in_maps = _host_prep(adj, u_feature, v_feature, weight_u, weight_v)
    res = bass_utils.run_bass_kernel_spmd(
        nc, in_maps, core_ids=list(range(N_CORES)), trace=_trace)
    _CACHE["last_result"] = res

    nz = adj > 0
    deg_u = nz.sum(axis=1).astype(np.float64)
    deg_v = nz.sum(axis=0).astype(np.float64)
    d_u = np.where(deg_u > 0, 1.0 / np.maximum(deg_u, 0.5), 0.0)
    d_v = np.where(deg_v > 0, 1.0 / np.maximum(deg_v, 0.5), 0.0)

    acc_u = np.concatenate(
        [(res.results[c]["out_ut"][0:64].astype(np.float32)
          + res.results[c]["out_ut"][64:128].astype(np.float32)).T
         for c in range(N_CORES)], axis=0)                   # [N_U, F]
    out_u = np.maximum(acc_u * d_u[:, None], 0.0).astype(np.float32)

    acc_v = np.zeros((128, N_V), np.float64)
    for c in range(N_CORES):
        acc_v += res.results[c]["out_vt"]
    acc_v = (acc_v[0:64] + acc_v[64:128]).T                  # [N_V, F]
    out_v = np.maximum(acc_v * d_v[:, None], 0.0).astype(np.float32)
    return out_u, out_v


# revision 9
# speedup vs baseline: 1.1254x; 1.1254x over previous
"""Bipartite graph convolution (GCMC-style) Trainium2 kernel, 8-core SPMD.

Math (reference): per-rating masks M_r = (adj == r), r=1..5,
  out_u = relu(d_u * sum_r (M_r @ v_feat) @ W_u[r]),  d_u = 1/deg_u
  out_v = relu(d_v * sum_r (M_r.T @ u_feat) @ W_v[r]), d_v = 1/deg_v

Device formulation (per core, u-rows sharded 1024/core), v2:
  Fold weights on host: P_r = v_feat @ W_u[r], Q_r = u_shard @ W_v[r].
  Step basis: {a, s2, s3, s4, s5} with s_c(a) = (a >= c); host solves
  P^_k so that sum_k g_k(a) P^_k == P_a for a in 0..5 (0 -> 0).
  The masks/steps are the MOVING matmul operand (long 512-col streams)
  and the 64-wide features are stationary; two independent M=64 matmuls
  run concurrently in the two column halves of the PE array
  (tile_position col groups), so the array is fully utilized:
    group A (psum partitions 0:64)  <- even chunk of the pair
    group B (psum partitions 64:128) <- odd chunk of the pair
  Host sums the two halves afterwards (plus cross-core all-reduce for
  out_v), applies 1/deg scaling and relu.
  Step planes are generated on-chip from the streamed adj tiles:
  DVE is_ge for most planes, ACT sigmoid(40*(a-c+0.5)) for ~1 plane per
  position to balance engine load (both give exact 0/1 in fp16).
"""

import numpy as np
import sys

sys.path.insert(0, "/opt/trn_rl_repo")

N_U, N_V = 8192, 8192
F = 64
R = 5
N_CORES = 8
U_SH = N_U // N_CORES          # 1024 rows per core
UC = U_SH // 128               # 8 u-chunks per core
VC = N_V // 128                # 64 v-chunks
WB = 2048                      # phase-B v-window width
NWIN = N_V // WB               # 4 windows

_CACHE = {}

# engine split: positions with (index % ACT_SKIP == ACT_SKIP-1) generate
# all 4 step planes on DVE; the rest do 3 on DVE + threshold-4 on ACT.
ACT_SKIP = 8


def _build():
    import concourse.bass as bass
    import concourse.bacc as bacc
    import concourse.mybir as mybir
    import concourse.tile as tile

    dt = mybir.dt
    ge = mybir.AluOpType.is_ge
    SIG = mybir.ActivationFunctionType.Sigmoid

    nc = bacc.Bacc("TRN2", target_bir_lowering=False, debug=False,
                   num_devices=N_CORES)

    adjt_h = nc.dram_tensor("adjt_h", [N_V, U_SH], dt.float16,
                            kind="ExternalInput").ap()
    adj_h = nc.dram_tensor("adj_h", [U_SH, N_V], dt.float16,
                           kind="ExternalInput").ap()
    p_hat_h = nc.dram_tensor("p_hat_h", [128, VC * R * F], dt.float16,
                             kind="ExternalInput").ap()
    q_hat_h = nc.dram_tensor("q_hat_h", [128, UC * R * F], dt.float16,
                             kind="ExternalInput").ap()
    out_ut = nc.dram_tensor("out_ut", [128, U_SH], dt.float16,
                            kind="ExternalOutput").ap()
    out_vt = nc.dram_tensor("out_vt", [128, N_V], dt.float16,
                            kind="ExternalOutput").ap()

    def gen_planes(nc, pl, src, W, all_dve, bias_s4):
        """Write step planes s2..s5 of src into quarters of pl [128, 4W].
        Threshold c=k+2 for quarter k. DVE is_ge for all but k=2, which
        goes to ACT (sigmoid step) unless all_dve."""
        for k in range(4):
            c = k + 2
            dst = pl[:, k * W:(k + 1) * W]
            if k == 2 and not all_dve:
                # sigmoid(40*(a - (c-0.5))): exact 0/1 in fp16 for ints
                nc.scalar.activation(dst, src, SIG,
                                     bias=bias_s4[:, 0:1], scale=40.0)
            else:
                nc.vector.tensor_scalar(dst, src, float(c), None, op0=ge)

    with tile.TileContext(nc) as tc:
        with tc.tile_pool(name="consts", bufs=1) as cons, \
             tc.tile_pool(name="streamA", bufs=4) as streamA, \
             tc.tile_pool(name="planesA", bufs=3) as planesA, \
             tc.tile_pool(name="streamB", bufs=4) as streamB, \
             tc.tile_pool(name="planesB", bufs=3) as planesB, \
             tc.tile_pool(name="fin", bufs=4) as fin:

            p_hat = cons.tile([128, VC * R * F], dt.float16, tag="ph")
            q_hat = cons.tile([128, UC * R * F], dt.float16, tag="qh")
            # chunked so early matmuls only wait for their slice
            PCH = VC * R * F // 16
            for i in range(16):
                nc.sync.dma_start(p_hat[:, i * PCH:(i + 1) * PCH],
                                  p_hat_h[:, i * PCH:(i + 1) * PCH])
            nc.sync.dma_start(q_hat[:], q_hat_h[:])
            bias_s4 = cons.tile([128, 1], dt.float32, tag="bs4")
            nc.gpsimd.memset(bias_s4[:], -40.0 * 3.5)

            # ---------------- Phase A: out_uT ----------------
            pspA = tc.tile_pool(name="psumA", bufs=1, space="PSUM")
            psA = pspA.__enter__()
            ps_u = psA.tile([128, U_SH], dt.float32, tag="psu")
            pos = 0
            for t in range(VC // 2):
                srcs = []
                planes = []
                for j in (0, 1):
                    vc = 2 * t + j
                    at = streamA.tile([128, U_SH], dt.float16, tag="adjt",
                                      name=f"at{vc}")
                    nc.sync.dma_start(at[:],
                                      adjt_h[vc * 128:(vc + 1) * 128, :])
                    pl = planesA.tile([128, 4 * U_SH], dt.float16,
                                      tag="plA", name=f"plA{vc}")
                    all_dve = (pos % ACT_SKIP) == ACT_SKIP - 1
                    pos += 1
                    gen_planes(nc, pl, at[:], U_SH, all_dve, bias_s4)
                    srcs.append(at)
                    planes.append(pl)
                for b in range(R):
                    for n in range(U_SH // 512):
                        for j in (0, 1):
                            vc = 2 * t + j
                            if b == 0:
                                mov = srcs[j][:, n * 512:(n + 1) * 512]
                            else:
                                off = (b - 1) * U_SH + n * 512
                                mov = planes[j][:, off:off + 512]
                            nc.tensor.matmul(
                                ps_u[64 * j:64 * (j + 1),
                                     n * 512:(n + 1) * 512],
                                p_hat[:, (vc * R + b) * F:
                                      (vc * R + b + 1) * F],
                                mov,
                                start=(t == 0 and b == 0),
                                stop=(t == VC // 2 - 1 and b == R - 1),
                                skip_group_check=True)
            # evacuate out_uT (raw; host applies deg/relu/transpose)
            for n in range(U_SH // 512):
                ev = fin.tile([128, 512], dt.float16, tag="evu",
                              name=f"evu{n}")
                if n % 2 == 0:
                    nc.scalar.copy(ev[:], ps_u[:, n * 512:(n + 1) * 512])
                else:
                    nc.vector.tensor_copy(ev[:], ps_u[:, n * 512:(n + 1) * 512])
                nc.sync.dma_start(out_ut[:, n * 512:(n + 1) * 512], ev[:])
            pspA.__exit__(None, None, None)

            # ---------------- Phase B: out_vT ----------------
            pspB = tc.tile_pool(name="psumB", bufs=1, space="PSUM")
            psB = pspB.__enter__()
            pos = 0
            for w in range(NWIN):
                psv = psB.tile([128, WB], dt.float32, tag="psv", bufs=2,
                               name=f"psv{w}")
                for up in range(UC // 2):
                    srcs = []
                    planes = []
                    for j in (0, 1):
                        uc = 2 * up + j
                        ad = streamB.tile([128, WB], dt.float16, tag="adjb",
                                          name=f"ad{w}_{uc}")
                        nc.sync.dma_start(
                            ad[:], adj_h[uc * 128:(uc + 1) * 128,
                                         w * WB:(w + 1) * WB])
                        pl = planesB.tile([128, 4 * WB], dt.float16,
                                          tag="plB", name=f"plB{w}_{uc}")
                        all_dve = (pos % ACT_SKIP) == ACT_SKIP - 1
                        pos += 1
                        gen_planes(nc, pl, ad[:], WB, all_dve, bias_s4)
                        srcs.append(ad)
                        planes.append(pl)
                    for b in range(R):
                        for n in range(WB // 512):
                            for j in (0, 1):
                                uc = 2 * up + j
                                if b == 0:
                                    mov = srcs[j][:, n * 512:(n + 1) * 512]
                                else:
                                    off = (b - 1) * WB + n * 512
                                    mov = planes[j][:, off:off + 512]
                                nc.tensor.matmul(
                                    psv[64 * j:64 * (j + 1),
                                        n * 512:(n + 1) * 512],
                                    q_hat[:, (uc * R + b) * F:
                                          (uc * R + b + 1) * F],
                                    mov,
                                    start=(up == 0 and b == 0),
                                    stop=(up == UC // 2 - 1 and b == R - 1),
                                    skip_group_check=True)
                for n in range(WB // 512):
                    ev = fin.tile([128, 512], dt.float16, tag="evv",
                                  name=f"evv{w}_{n}")
                    if n % 2 == 0:
                        nc.scalar.copy(ev[:], psv[:, n * 512:(n + 1) * 512])
                    else:
                        nc.vector.tensor_copy(ev[:],
                                              psv[:, n * 512:(n + 1) * 512])
                    nc.sync.dma_start(
                        out_vt[:, w * WB + n * 512:w * WB + (n + 1) * 512],
                        ev[:])
            pspB.__exit__(None, None, None)

    nc.compile()
    return nc


def _basis_matrix():
    # rows r=1..5, cols k: [a, s2, s3, s4, s5]
    M = np.zeros((R, R))
    for r in range(1, R + 1):
        M[r - 1, 0] = r
        for k in range(1, R):
            M[r - 1, k] = 1.0 if r >= k + 1 else 0.0
    return M


def _host_prep(adj, u_feature, v_feature, weight_u, weight_v):
    adj = np.asarray(adj)
    u_feature = np.asarray(u_feature, dtype=np.float32)
    v_feature = np.asarray(v_feature, dtype=np.float32)
    weight_u = np.asarray(weight_u, dtype=np.float32)
    weight_v = np.asarray(weight_v, dtype=np.float32)

    adj16 = adj.astype(np.float16)
    Minv = np.linalg.inv(_basis_matrix())

    # P_r = v_feat @ W_u[r]; basis-transform to step basis
    P = np.einsum("vf,rfo->rvo", v_feature, weight_u)       # [R, N_V, F]
    Pb = np.tensordot(Minv, P, axes=([1], [0]))             # [R, N_V, F]
    # p_hat[p, (vc*R+b)*F + f] = Pb[b, vc*128+p, f]
    p_hat = np.ascontiguousarray(
        Pb.reshape(R, VC, 128, F).transpose(2, 1, 0, 3).reshape(128, -1)
    ).astype(np.float16)

    in_maps = []
    for c in range(N_CORES):
        sl = slice(c * U_SH, (c + 1) * U_SH)
        Q = np.einsum("uf,rfo->ruo", u_feature[sl], weight_v)  # [R, U_SH, F]
        Qb = np.tensordot(Minv, Q, axes=([1], [0]))
        q_hat = np.ascontiguousarray(
            Qb.reshape(R, UC, 128, F).transpose(2, 1, 0, 3).reshape(128, -1)
        ).astype(np.float16)
        a = adj16[sl]
        in_maps.append({
            "adj_h": np.ascontiguousarray(a),
            "adjt_h": np.ascontiguousarray(a.T),
            "p_hat_h": p_hat,
            "q_hat_h": q_hat,
        })
    return in_maps


def kernel(adj, u_feature, v_feature, weight_u, weight_v, _trace=False):
    from concourse import bass_utils

    if "nc" not in _CACHE:
        _CACHE["nc"] = _build()
    nc = _CACHE["nc"]

    adj = np.asarray(adj)
    in_maps = _host_prep(adj, u_feature, v_feature, weight_u, weight_v)
    res = bass_utils.run_bass_kernel_spmd(
        nc, in_maps, core_ids=list(range(N_CORES)), trace=_trace)
    _CACHE["last_result"] = res

    nz = adj > 0
    deg_u = nz.sum(axis=1).astype(np.float64)
    deg_v = nz.sum(axis=0).astype(np.float64)
    d_u = np.where(deg_u > 0, 1.0 / np.maximum(deg_u, 0.5), 0.0)
    d_v = np.where(deg_v > 0, 1.0 / np.maximum(deg_v, 0.5), 0.0)

    acc_u = np.concatenate(
        [(res.results[c]["out_ut"][0:64].astype(np.float32)
          + res.results[c]["out_ut"][64:128].astype(np.float32)).T
         for c in range(N_CORES)], axis=0)                   # [N_U, F]
    out_u = np.maximum(acc_u * d_u[:, None], 0.0).astype(np.float32)

    acc_v = np.zeros((128, N_V), np.float64)
    for c in range(N_CORES):
        acc_v += res.results[c]["out_vt"]
    acc_v = (acc_v[0:64] + acc_v[64:128]).T                  # [N_V, F]
    out_v = np.maximum(acc_v * d_v[:, None], 0.0).astype(np.float32)
    return out_u, out_v


# revision 10
# speedup vs baseline: 1.2434x; 1.1049x over previous
"""Bipartite graph convolution (GCMC-style) Trainium2 kernel, 8-core SPMD.

Math (reference): per-rating masks M_r = (adj == r), r=1..5,
  out_u = relu(d_u * sum_r (M_r @ v_feat) @ W_u[r]),  d_u = 1/deg_u
  out_v = relu(d_v * sum_r (M_r.T @ u_feat) @ W_v[r]), d_v = 1/deg_v

Device formulation (per core, u-rows sharded 1024/core), v2:
  Fold weights on host: P_r = v_feat @ W_u[r], Q_r = u_shard @ W_v[r].
  Step basis: {a, s2, s3, s4, s5} with s_c(a) = (a >= c); host solves
  P^_k so that sum_k g_k(a) P^_k == P_a for a in 0..5 (0 -> 0).
  The masks/steps are the MOVING matmul operand (long 512-col streams)
  and the 64-wide features are stationary; two independent M=64 matmuls
  run concurrently in the two column halves of the PE array
  (tile_position col groups), so the array is fully utilized:
    group A (psum partitions 0:64)  <- even chunk of the pair
    group B (psum partitions 64:128) <- odd chunk of the pair
  Host sums the two halves afterwards (plus cross-core all-reduce for
  out_v), applies 1/deg scaling and relu.
  Step planes are generated on-chip from the streamed adj tiles:
  DVE is_ge for most planes, ACT sigmoid(40*(a-c+0.5)) for ~1 plane per
  position to balance engine load (both give exact 0/1 in fp16).
"""

import numpy as np
import sys

sys.path.insert(0, "/opt/trn_rl_repo")

N_U, N_V = 8192, 8192
F = 64
R = 5
N_CORES = 8
U_SH = N_U // N_CORES          # 1024 rows per core
UC = U_SH // 128               # 8 u-chunks per core
VC = N_V // 128                # 64 v-chunks
WB = 2048                      # phase-B v-window width
NWIN = N_V // WB               # 4 windows

_CACHE = {}

# engine split: positions with (index % ACT_SKIP == ACT_SKIP-1) generate
# all 4 step planes on DVE; the rest do 3 on DVE + threshold-4 on ACT.
ACT_SKIP = 8


def _build():
    import concourse.bass as bass
    import concourse.bacc as bacc
    import concourse.mybir as mybir
    import concourse.tile as tile

    dt = mybir.dt
    ge = mybir.AluOpType.is_ge
    SIG = mybir.ActivationFunctionType.Sigmoid

    nc = bacc.Bacc("TRN2", target_bir_lowering=False, debug=False,
                   num_devices=N_CORES)

    adjt_h = nc.dram_tensor("adjt_h", [N_V, U_SH], dt.float16,
                            kind="ExternalInput").ap()
    adj_h = nc.dram_tensor("adj_h", [U_SH, N_V], dt.float16,
                           kind="ExternalInput").ap()
    p_hat_h = nc.dram_tensor("p_hat_h", [128, VC * R * F], dt.float16,
                             kind="ExternalInput").ap()
    q_hat_h = nc.dram_tensor("q_hat_h", [128, UC * R * F], dt.float16,
                             kind="ExternalInput").ap()
    out_ut = nc.dram_tensor("out_ut", [128, U_SH], dt.float16,
                            kind="ExternalOutput").ap()
    out_vt = nc.dram_tensor("out_vt", [128, N_V], dt.float16,
                            kind="ExternalOutput").ap()

    def gen_planes(nc, pl, src, W, all_dve, bias_s4):
        """Write step planes s2..s5 of src into quarters of pl [128, 4W].
        Threshold c=k+2 for quarter k. DVE is_ge for all but k=2, which
        goes to ACT (sigmoid step) unless all_dve."""
        for k in range(4):
            c = k + 2
            dst = pl[:, k * W:(k + 1) * W]
            if k == 2 and not all_dve:
                # sigmoid(40*(a - (c-0.5))): exact 0/1 in fp16 for ints
                nc.scalar.activation(dst, src, SIG,
                                     bias=bias_s4[:, 0:1], scale=40.0)
            else:
                nc.vector.tensor_scalar(dst, src, float(c), None, op0=ge)

    with tile.TileContext(nc) as tc:
        with tc.tile_pool(name="consts", bufs=1) as cons, \
             tc.tile_pool(name="streamA", bufs=6) as streamA, \
             tc.tile_pool(name="planesA", bufs=3) as planesA, \
             tc.tile_pool(name="streamB", bufs=6) as streamB, \
             tc.tile_pool(name="planesB", bufs=3) as planesB, \
             tc.tile_pool(name="fin", bufs=4) as fin:

            p_hat = cons.tile([128, VC * R * F], dt.float16, tag="ph")
            q_hat = cons.tile([128, UC * R * F], dt.float16, tag="qh")
            # p_hat is DMA'd in 16 chunks (4 vc each) interleaved with the
            # adjT stream below so early matmuls only wait for their slice
            PCH = VC * R * F // 16

            def p_chunk(i):
                nc.sync.dma_start(p_hat[:, i * PCH:(i + 1) * PCH],
                                  p_hat_h[:, i * PCH:(i + 1) * PCH])
            bias_s4 = cons.tile([128, 1], dt.float32, tag="bs4")
            nc.gpsimd.memset(bias_s4[:], -40.0 * 3.5)

            # ---------------- Phase A: out_uT ----------------
            pspA = tc.tile_pool(name="psumA", bufs=1, space="PSUM")
            psA = pspA.__enter__()
            ps_u = psA.tile([128, U_SH], dt.float32, tag="psu")
            pos = 0
            for i in range(3):
                p_chunk(i)
            for t in range(VC // 2):
                ch = t // 2 + 3
                if t % 2 == 0 and ch < 16:
                    p_chunk(ch)
                if t == 20:
                    nc.sync.dma_start(q_hat[:], q_hat_h[:])
                srcs = []
                planes = []
                for j in (0, 1):
                    vc = 2 * t + j
                    at = streamA.tile([128, U_SH], dt.float16, tag="adjt",
                                      name=f"at{vc}")
                    nc.sync.dma_start(at[:],
                                      adjt_h[vc * 128:(vc + 1) * 128, :])
                    pl = planesA.tile([128, 4 * U_SH], dt.float16,
                                      tag="plA", name=f"plA{vc}")
                    all_dve = (pos % ACT_SKIP) == ACT_SKIP - 1
                    pos += 1
                    gen_planes(nc, pl, at[:], U_SH, all_dve, bias_s4)
                    srcs.append(at)
                    planes.append(pl)
                for b in range(R):
                    for n in range(U_SH // 512):
                        for j in (0, 1):
                            vc = 2 * t + j
                            if b == 0:
                                mov = srcs[j][:, n * 512:(n + 1) * 512]
                            else:
                                off = (b - 1) * U_SH + n * 512
                                mov = planes[j][:, off:off + 512]
                            nc.tensor.matmul(
                                ps_u[64 * j:64 * (j + 1),
                                     n * 512:(n + 1) * 512],
                                p_hat[:, (vc * R + b) * F:
                                      (vc * R + b + 1) * F],
                                mov,
                                start=(t == 0 and b == 0),
                                stop=(t == VC // 2 - 1 and b == R - 1),
                                skip_group_check=True)
            # evacuate out_uT (raw; host applies deg/relu/transpose)
            for n in range(U_SH // 512):
                ev = fin.tile([128, 512], dt.float16, tag="evu",
                              name=f"evu{n}")
                if n % 2 == 0:
                    nc.scalar.copy(ev[:], ps_u[:, n * 512:(n + 1) * 512])
                else:
                    nc.vector.tensor_copy(ev[:], ps_u[:, n * 512:(n + 1) * 512])
                nc.sync.dma_start(out_ut[:, n * 512:(n + 1) * 512], ev[:])
            pspA.__exit__(None, None, None)

            # ---------------- Phase B: out_vT ----------------
            pspB = tc.tile_pool(name="psumB", bufs=1, space="PSUM")
            psB = pspB.__enter__()
            pos = 0
            for w in range(NWIN):
                psv = psB.tile([128, WB], dt.float32, tag="psv", bufs=2,
                               name=f"psv{w}")
                for up in range(UC // 2):
                    srcs = []
                    planes = []
                    for j in (0, 1):
                        uc = 2 * up + j
                        ad = streamB.tile([128, WB], dt.float16, tag="adjb",
                                          name=f"ad{w}_{uc}")
                        nc.sync.dma_start(
                            ad[:], adj_h[uc * 128:(uc + 1) * 128,
                                         w * WB:(w + 1) * WB])
                        pl = planesB.tile([128, 4 * WB], dt.float16,
                                          tag="plB", name=f"plB{w}_{uc}")
                        all_dve = (pos % ACT_SKIP) == ACT_SKIP - 1
                        pos += 1
                        gen_planes(nc, pl, ad[:], WB, all_dve, bias_s4)
                        srcs.append(ad)
                        planes.append(pl)
                    for b in range(R):
                        for n in range(WB // 512):
                            for j in (0, 1):
                                uc = 2 * up + j
                                if b == 0:
                                    mov = srcs[j][:, n * 512:(n + 1) * 512]
                                else:
                                    off = (b - 1) * WB + n * 512
                                    mov = planes[j][:, off:off + 512]
                                nc.tensor.matmul(
                                    psv[64 * j:64 * (j + 1),
                                        n * 512:(n + 1) * 512],
                                    q_hat[:, (uc * R + b) * F:
                                          (uc * R + b + 1) * F],
                                    mov,
                                    start=(up == 0 and b == 0),
                                    stop=(up == UC // 2 - 1 and b == R - 1),
                                    skip_group_check=True)
                for n in range(WB // 512):
                    ev = fin.tile([128, 512], dt.float16, tag="evv",
                                  name=f"evv{w}_{n}")
                    if n % 2 == 0:
                        nc.scalar.copy(ev[:], psv[:, n * 512:(n + 1) * 512])
                    else:
                        nc.vector.tensor_copy(ev[:],
                                              psv[:, n * 512:(n + 1) * 512])
                    nc.sync.dma_start(
                        out_vt[:, w * WB + n * 512:w * WB + (n + 1) * 512],
                        ev[:])
            pspB.__exit__(None, None, None)

    nc.compile()
    return nc


def _basis_matrix():
    # rows r=1..5, cols k: [a, s2, s3, s4, s5]
    M = np.zeros((R, R))
    for r in range(1, R + 1):
        M[r - 1, 0] = r
        for k in range(1, R):
            M[r - 1, k] = 1.0 if r >= k + 1 else 0.0
    return M


def _host_prep(adj, u_feature, v_feature, weight_u, weight_v):
    adj = np.asarray(adj)
    u_feature = np.asarray(u_feature, dtype=np.float32)
    v_feature = np.asarray(v_feature, dtype=np.float32)
    weight_u = np.asarray(weight_u, dtype=np.float32)
    weight_v = np.asarray(weight_v, dtype=np.float32)

    adj16 = adj.astype(np.float16)
    Minv = np.linalg.inv(_basis_matrix())

    # P_r = v_feat @ W_u[r]; basis-transform to step basis
    P = np.einsum("vf,rfo->rvo", v_feature, weight_u)       # [R, N_V, F]
    Pb = np.tensordot(Minv, P, axes=([1], [0]))             # [R, N_V, F]
    # p_hat[p, (vc*R+b)*F + f] = Pb[b, vc*128+p, f]
    p_hat = np.ascontiguousarray(
        Pb.reshape(R, VC, 128, F).transpose(2, 1, 0, 3).reshape(128, -1)
    ).astype(np.float16)

    in_maps = []
    for c in range(N_CORES):
        sl = slice(c * U_SH, (c + 1) * U_SH)
        Q = np.einsum("uf,rfo->ruo", u_feature[sl], weight_v)  # [R, U_SH, F]
        Qb = np.tensordot(Minv, Q, axes=([1], [0]))
        q_hat = np.ascontiguousarray(
            Qb.reshape(R, UC, 128, F).transpose(2, 1, 0, 3).reshape(128, -1)
        ).astype(np.float16)
        a = adj16[sl]
        in_maps.append({
            "adj_h": np.ascontiguousarray(a),
            "adjt_h": np.ascontiguousarray(a.T),
            "p_hat_h": p_hat,
            "q_hat_h": q_hat,
        })
    return in_maps


def kernel(adj, u_feature, v_feature, weight_u, weight_v, _trace=False):
    from concourse import bass_utils

    if "nc" not in _CACHE:
        _CACHE["nc"] = _build()
    nc = _CACHE["nc"]

    adj = np.asarray(adj)
    in_maps = _host_prep(adj, u_feature, v_feature, weight_u, weight_v)
    res = bass_utils.run_bass_kernel_spmd(
        nc, in_maps, core_ids=list(range(N_CORES)), trace=_trace)
    _CACHE["last_result"] = res

    nz = adj > 0
    deg_u = nz.sum(axis=1).astype(np.float64)
    deg_v = nz.sum(axis=0).astype(np.float64)
    d_u = np.where(deg_u > 0, 1.0 / np.maximum(deg_u, 0.5), 0.0)
    d_v = np.where(deg_v > 0, 1.0 / np.maximum(deg_v, 0.5), 0.0)

    acc_u = np.concatenate(
        [(res.results[c]["out_ut"][0:64].astype(np.float32)
          + res.results[c]["out_ut"][64:128].astype(np.float32)).T
         for c in range(N_CORES)], axis=0)                   # [N_U, F]
    out_u = np.maximum(acc_u * d_u[:, None], 0.0).astype(np.float32)

    acc_v = np.zeros((128, N_V), np.float64)
    for c in range(N_CORES):
        acc_v += res.results[c]["out_vt"]
    acc_v = (acc_v[0:64] + acc_v[64:128]).T                  # [N_V, F]
    out_v = np.maximum(acc_v * d_v[:, None], 0.0).astype(np.float32)
    return out_u, out_v


# revision 21
# speedup vs baseline: 1.3034x; 1.0482x over previous
"""Bipartite graph convolution (GCMC-style) Trainium2 kernel, 8-core SPMD.

Math (reference): per-rating masks M_r = (adj == r), r=1..5,
  out_u = relu(d_u * sum_r (M_r @ v_feat) @ W_u[r]),  d_u = 1/deg_u
  out_v = relu(d_v * sum_r (M_r.T @ u_feat) @ W_v[r]), d_v = 1/deg_v

Device formulation (per core, u-rows sharded 1024/core), v2:
  Fold weights on host: P_r = v_feat @ W_u[r], Q_r = u_shard @ W_v[r].
  Step basis: {a, s2, s3, s4, s5} with s_c(a) = (a >= c); host solves
  P^_k so that sum_k g_k(a) P^_k == P_a for a in 0..5 (0 -> 0).
  The masks/steps are the MOVING matmul operand (long 512-col streams)
  and the 64-wide features are stationary; two independent M=64 matmuls
  run concurrently in the two column halves of the PE array
  (tile_position col groups), so the array is fully utilized:
    group A (psum partitions 0:64)  <- even chunk of the pair
    group B (psum partitions 64:128) <- odd chunk of the pair
  Host sums the two halves afterwards (plus cross-core all-reduce for
  out_v), applies 1/deg scaling and relu.
  Step planes are generated on-chip from the streamed adj tiles:
  DVE is_ge for most planes, ACT sigmoid(40*(a-c+0.5)) for ~1 plane per
  position to balance engine load (both give exact 0/1 in fp16).
"""

import numpy as np
import sys

sys.path.insert(0, "/opt/trn_rl_repo")

N_U, N_V = 8192, 8192
F = 64
R = 5
N_CORES = 8
U_SH = N_U // N_CORES          # 1024 rows per core
UC = U_SH // 128               # 8 u-chunks per core
VC = N_V // 128                # 64 v-chunks
WB = 2048                      # phase-B v-window width
NWIN = N_V // WB               # 4 windows

_CACHE = {}

# engine split: positions with (index % ACT_SKIP == ACT_SKIP-1) generate
# all 4 step planes on DVE; the rest do 3 on DVE + threshold-4 on ACT.
ACT_SKIP = 8


def _build():
    import concourse.bass as bass
    import concourse.bacc as bacc
    import concourse.mybir as mybir
    import concourse.tile as tile

    dt = mybir.dt
    ge = mybir.AluOpType.is_ge
    SIG = mybir.ActivationFunctionType.Sigmoid

    nc = bacc.Bacc("TRN2", target_bir_lowering=False, debug=False,
                   num_devices=N_CORES)

    adjt_h = nc.dram_tensor("adjt_h", [N_V, U_SH], dt.float16,
                            kind="ExternalInput").ap()
    adj_h = nc.dram_tensor("adj_h", [U_SH, N_V], dt.float16,
                           kind="ExternalInput").ap()
    p_hat_h = nc.dram_tensor("p_hat_h", [128, VC * R * F], dt.float16,
                             kind="ExternalInput").ap()
    q_hat_h = nc.dram_tensor("q_hat_h", [128, UC * R * F], dt.float16,
                             kind="ExternalInput").ap()
    out_ut = nc.dram_tensor("out_ut", [128, U_SH], dt.float16,
                            kind="ExternalOutput").ap()
    out_vt = nc.dram_tensor("out_vt", [128, N_V], dt.float16,
                            kind="ExternalOutput").ap()

    def gen_planes(nc, pl, src, W, all_dve, bias_s4):
        """Write step planes s2..s5 of src into quarters of pl [128, 4W].
        Threshold c=k+2 for quarter k. DVE is_ge for all but k=2, which
        goes to ACT (sigmoid step) unless all_dve."""
        for k in range(4):
            c = k + 2
            dst = pl[:, k * W:(k + 1) * W]
            if k == 2 and not all_dve:
                # sigmoid(40*(a - (c-0.5))): exact 0/1 in fp16 for ints
                nc.scalar.activation(dst, src, SIG,
                                     bias=bias_s4[:, 0:1], scale=40.0)
            else:
                nc.vector.tensor_scalar(dst, src, float(c), None, op0=ge)

    with tile.TileContext(nc) as tc:
        with tc.tile_pool(name="consts", bufs=1) as cons, \
             tc.tile_pool(name="streamA", bufs=4) as streamA, \
             tc.tile_pool(name="planesA", bufs=3) as planesA, \
             tc.tile_pool(name="streamB", bufs=6) as streamB, \
             tc.tile_pool(name="planesB", bufs=3) as planesB, \
             tc.tile_pool(name="fin", bufs=4) as fin:

            p_hat = cons.tile([128, VC * R * F], dt.float16, tag="ph")
            q_hat = cons.tile([128, UC * R * F], dt.float16, tag="qh")
            # p_hat is DMA'd in 16 chunks (4 vc each) interleaved with the
            # adjT stream below so early matmuls only wait for their slice
            PCH = VC * R * F // 16

            def p_chunk(i):
                nc.sync.dma_start(p_hat[:, i * PCH:(i + 1) * PCH],
                                  p_hat_h[:, i * PCH:(i + 1) * PCH])
            bias_s4 = cons.tile([128, 1], dt.float32, tag="bs4")
            nc.gpsimd.memset(bias_s4[:], -40.0 * 3.5)
            zwarm = cons.tile([128, 512], dt.float16, tag="zw")
            nc.gpsimd.memset(zwarm[:], 0.0)

            # ---------------- Phase A: out_uT ----------------
            pspA = tc.tile_pool(name="psumA", bufs=1, space="PSUM")
            psA = pspA.__enter__()
            ps_u = psA.tile([128, U_SH], dt.float32, tag="psu")
            # dependency-free warmup matmuls: run during the startup DMA
            # window so the PE HAM clock-gate is at 8/8 when real work lands
            pwarm = psA.tile([128, 512], dt.float32, tag="pwarm")
            for i in range(10):
                nc.tensor.matmul(pwarm[:], zwarm[:, 0:128], zwarm[:],
                                 start=(i == 0), stop=(i == 9),
                                 skip_group_check=True)
            pos = 0
            for i in range(3):
                p_chunk(i)
            for t in range(VC // 2):
                ch = t // 2 + 3
                if t % 2 == 0 and ch < 16:
                    p_chunk(ch)
                if t == 20:
                    nc.sync.dma_start(q_hat[:], q_hat_h[:])
                srcs = []
                planes = []
                for j in (0, 1):
                    vc = 2 * t + j
                    at = streamA.tile([128, U_SH], dt.float16, tag="adjt",
                                      name=f"at{vc}")
                    nc.sync.dma_start(at[:],
                                      adjt_h[vc * 128:(vc + 1) * 128, :])
                    pl = planesA.tile([128, 4 * U_SH], dt.float16,
                                      tag="plA", name=f"plA{vc}")
                    all_dve = (pos % ACT_SKIP) == ACT_SKIP - 1
                    pos += 1
                    gen_planes(nc, pl, at[:], U_SH, all_dve, bias_s4)
                    srcs.append(at)
                    planes.append(pl)
                for b in range(R):
                    for n in range(U_SH // 512):
                        for j in (0, 1):
                            vc = 2 * t + j
                            if b == 0:
                                mov = srcs[j][:, n * 512:(n + 1) * 512]
                            else:
                                off = (b - 1) * U_SH + n * 512
                                mov = planes[j][:, off:off + 512]
                            nc.tensor.matmul(
                                ps_u[64 * j:64 * (j + 1),
                                     n * 512:(n + 1) * 512],
                                p_hat[:, (vc * R + b) * F:
                                      (vc * R + b + 1) * F],
                                mov,
                                start=(t == 0 and b == 0),
                                stop=(t == VC // 2 - 1 and b == R - 1),
                                skip_group_check=True)
            # evacuate out_uT (raw; host applies deg/relu/transpose)
            for n in range(U_SH // 512):
                ev = fin.tile([128, 512], dt.float16, tag="evu",
                              name=f"evu{n}")
                if n % 2 == 0:
                    nc.scalar.copy(ev[:], ps_u[:, n * 512:(n + 1) * 512])
                else:
                    nc.vector.tensor_copy(ev[:], ps_u[:, n * 512:(n + 1) * 512])
                nc.sync.dma_start(out_ut[:, n * 512:(n + 1) * 512], ev[:])
            pspA.__exit__(None, None, None)

            # ---------------- Phase B: out_vT ----------------
            pspB = tc.tile_pool(name="psumB", bufs=1, space="PSUM")
            psB = pspB.__enter__()
            pos = 0
            for w in range(NWIN):
                psv = psB.tile([128, WB], dt.float32, tag="psv", bufs=2,
                               name=f"psv{w}")
                for up in range(UC // 2):
                    srcs = []
                    planes = []
                    for j in (0, 1):
                        uc = 2 * up + j
                        ad = streamB.tile([128, WB], dt.float16, tag="adjb",
                                          name=f"ad{w}_{uc}")
                        nc.sync.dma_start(
                            ad[:], adj_h[uc * 128:(uc + 1) * 128,
                                         w * WB:(w + 1) * WB])
                        pl = planesB.tile([128, 4 * WB], dt.float16,
                                          tag="plB", name=f"plB{w}_{uc}")
                        all_dve = (pos % ACT_SKIP) == ACT_SKIP - 1
                        pos += 1
                        gen_planes(nc, pl, ad[:], WB, all_dve, bias_s4)
                        srcs.append(ad)
                        planes.append(pl)
                    for b in range(R):
                        for n in range(WB // 512):
                            for j in (0, 1):
                                uc = 2 * up + j
                                if b == 0:
                                    mov = srcs[j][:, n * 512:(n + 1) * 512]
                                else:
                                    off = (b - 1) * WB + n * 512
                                    mov = planes[j][:, off:off + 512]
                                nc.tensor.matmul(
                                    psv[64 * j:64 * (j + 1),
                                        n * 512:(n + 1) * 512],
                                    q_hat[:, (uc * R + b) * F:
                                          (uc * R + b + 1) * F],
                                    mov,
                                    start=(up == 0 and b == 0),
                                    stop=(up == UC // 2 - 1 and b == R - 1),
                                    skip_group_check=True)
                for n in range(WB // 512):
                    ev = fin.tile([128, 512], dt.float16, tag="evv",
                                  name=f"evv{w}_{n}")
                    if n % 2 == 0:
                        nc.scalar.copy(ev[:], psv[:, n * 512:(n + 1) * 512])
                    else:
                        nc.vector.tensor_copy(ev[:],
                                              psv[:, n * 512:(n + 1) * 512])
                    nc.sync.dma_start(
                        out_vt[:, w * WB + n * 512:w * WB + (n + 1) * 512],
                        ev[:])
            pspB.__exit__(None, None, None)

    nc.compile()
    return nc


def _basis_matrix():
    # rows r=1..5, cols k: [a, s2, s3, s4, s5]
    M = np.zeros((R, R))
    for r in range(1, R + 1):
        M[r - 1, 0] = r
        for k in range(1, R):
            M[r - 1, k] = 1.0 if r >= k + 1 else 0.0
    return M


def _host_prep(adj, u_feature, v_feature, weight_u, weight_v):
    adj = np.asarray(adj)
    u_feature = np.asarray(u_feature, dtype=np.float32)
    v_feature = np.asarray(v_feature, dtype=np.float32)
    weight_u = np.asarray(weight_u, dtype=np.float32)
    weight_v = np.asarray(weight_v, dtype=np.float32)

    adj16 = adj.astype(np.float16)
    Minv = np.linalg.inv(_basis_matrix())

    # P_r = v_feat @ W_u[r]; basis-transform to step basis
    P = np.einsum("vf,rfo->rvo", v_feature, weight_u)       # [R, N_V, F]
    Pb = np.tensordot(Minv, P, axes=([1], [0]))             # [R, N_V, F]
    # p_hat[p, (vc*R+b)*F + f] = Pb[b, vc*128+p, f]
    p_hat = np.ascontiguousarray(
        Pb.reshape(R, VC, 128, F).transpose(2, 1, 0, 3).reshape(128, -1)
    ).astype(np.float16)

    in_maps = []
    for c in range(N_CORES):
        sl = slice(c * U_SH, (c + 1) * U_SH)
        Q = np.einsum("uf,rfo->ruo", u_feature[sl], weight_v)  # [R, U_SH, F]
        Qb = np.tensordot(Minv, Q, axes=([1], [0]))
        q_hat = np.ascontiguousarray(
            Qb.reshape(R, UC, 128, F).transpose(2, 1, 0, 3).reshape(128, -1)
        ).astype(np.float16)
        a = adj16[sl]
        in_maps.append({
            "adj_h": np.ascontiguousarray(a),
            "adjt_h": np.ascontiguousarray(a.T),
            "p_hat_h": p_hat,
            "q_hat_h": q_hat,
        })
    return in_maps


def kernel(adj, u_feature, v_feature, weight_u, weight_v, _trace=False):
    from concourse import bass_utils

    if "nc" not in _CACHE:
        _CACHE["nc"] = _build()
    nc = _CACHE["nc"]

    adj = np.asarray(adj)
    in_maps = _host_prep(adj, u_feature, v_feature, weight_u, weight_v)
    res = bass_utils.run_bass_kernel_spmd(
        nc, in_maps, core_ids=list(range(N_CORES)), trace=_trace)
    _CACHE["last_result"] = res

    nz = adj > 0
    deg_u = nz.sum(axis=1).astype(np.float64)
    deg_v = nz.sum(axis=0).astype(np.float64)
    d_u = np.where(deg_u > 0, 1.0 / np.maximum(deg_u, 0.5), 0.0)
    d_v = np.where(deg_v > 0, 1.0 / np.maximum(deg_v, 0.5), 0.0)

    acc_u = np.concatenate(
        [(res.results[c]["out_ut"][0:64].astype(np.float32)
          + res.results[c]["out_ut"][64:128].astype(np.float32)).T
         for c in range(N_CORES)], axis=0)                   # [N_U, F]
    out_u = np.maximum(acc_u * d_u[:, None], 0.0).astype(np.float32)

    acc_v = np.zeros((128, N_V), np.float64)
    for c in range(N_CORES):
        acc_v += res.results[c]["out_vt"]
    acc_v = (acc_v[0:64] + acc_v[64:128]).T                  # [N_V, F]
    out_v = np.maximum(acc_v * d_v[:, None], 0.0).astype(np.float32)
    return out_u, out_v
